# revision 9
# baseline (speedup 1.0000x reference)
"""COLoRALinear fused kernel for 8 trn2 NeuronCores (Bass/Tile).

Problem: out = x@W.T + b + cw*2*(x@sA.T)@sB.T + (1-cw)*2*sum_t r[b,t]*(x@tA[t].T)@tB[t].T
with routing r = softmax(mean_s(x) @ emb.T), cw = sigmoid(collab_weight).

Sharding: core i -> batch element p=i//2 (2048 tokens), DOUT half h=i%2
(2048 cols). Each core holds its full batch element, so routing is local;
no collectives.

Device plan per core:
  - preload x^T as bf16, SBUF-resident ([128, 32, 2048], 128KB/part)
  - phase A: hid^T[80, 2048] = A_cat @ x^T  (A_cat rows: 8 shared + 64 task
    + 8 task_emb), evict rows 0..71 to bf16, free-reduce rows 72..79 into
    routing logits
  - routing: softmax on one partition, build svec[73] (cw2 / routing-scaled
    / 1.0-for-bias), scale B_cat rows -> bf16
  - main loop: 8 n-chunks of 256 cols; W^T k-tiles streamed fp32->bf16;
    per (n,m): 32 accumulating matmuls + 1 LoRA down-proj matmul into the
    same PSUM bank; evict fp32; store.
"""
import numpy as np
from contextlib import ExitStack

import concourse.bass as bass
import concourse.tile as tile
from concourse import mybir
from concourse.bass_utils import run_bass_kernel_spmd
from concourse.vector_clock import ScopedClock

B, S, DIN, DOUT, R, T = 4, 2048, 4096, 4096, 8, 8
SCALING = 2.0
N_CORES = 8
P = 128
KT = DIN // P            # 32 k-tiles
S_CORE = S               # tokens per core (one batch element)
N_CORE = DOUT // 2       # dout columns per core
NCH = 256                # n-chunk width
NNC = N_CORE // NCH      # 8 n-chunks
MT = S_CORE // P         # 16 m-tiles
AROWS = 80               # 8 shared + 64 task + 8 emb rows in A_cat
HID = 73                 # 72 lora rows + ones(bias) row
F32 = mybir.dt.float32
BF16 = mybir.dt.bfloat16


class _DrainSplitTileContext(tile.TileContext):
    """Walrus in this container rejects a Drain carrying >1 sem wait (the
    CTRL_NO encoding has one TPB_EVENTS wait slot). Split the exit drain's
    waits across a chain of single-wait drains."""

    def _drain_and_barrier(self, tick_clock, wait_clock):
        drain_inst = self.nc.sync.drain()
        wait_clock.add_sem_waits(
            drain_inst.ins, ScopedClock({None: tick_clock.global_clock})
        )
        si = drain_inst.ins.sync_info
        if si is not None and len(si.on_wait) > 1:
            waits = list(si.on_wait)
            drain_inst.ins.sync_info = mybir.SyncInfo(
                on_wait=[waits[0]], on_update=list(si.on_update)
            )
            for w in waits[1:]:
                extra = self.nc.sync.drain()
                extra.ins.sync_info = mybir.SyncInfo(on_wait=[w], on_update=[])

        self.nc.all_engine_barrier()
        assert self.sems is not None
        popped = self.nc._tile_sem_poison_stack.pop()
        assert popped is self._sem_poison
        self.nc.clear_and_free_semaphores(list(self.sems.allocated().values()))
        self.nc.all_engine_barrier()


_wsplit_counter = [0]


def _split_multi_waits(nc):
    """Walrus here lowers DMA/CTRL instructions with a single TPB_EVENTS wait
    slot and rejects >1 sem wait. Hoist extra waits onto same-engine NoOps
    inserted immediately before the offending instruction (engine program
    order makes this semantics-preserving)."""
    for f in nc.m.functions:
        for blk in f.blocks:
            insts = blk.instructions
            out = []
            changed = False
            for inst in insts:
                si = inst.sync_info
                if si is not None and len(si.on_wait) > 1:
                    waits = list(si.on_wait)
                    for w in waits[:-1]:
                        _wsplit_counter[0] += 1
                        nop = mybir.InstNoOp(name=f"I-wsplit-{_wsplit_counter[0]}")
                        nop.engine = inst.engine
                        nop.sync_info = mybir.SyncInfo(on_wait=[w], on_update=[])
                        out.append(nop)
                    inst.sync_info = mybir.SyncInfo(
                        on_wait=[waits[-1]], on_update=list(si.on_update)
                    )
                    changed = True
                out.append(inst)
            if changed:
                blk.instructions = out


def build_nc():
    nc = bass.Bass(trn_type="TRN2", target_bir_lowering=False)
    xt = nc.dram_tensor("xt", [DIN, S_CORE], F32, kind="ExternalInput").ap()
    wt = nc.dram_tensor("wt", [DIN, N_CORE], F32, kind="ExternalInput").ap()
    act = nc.dram_tensor("act", [KT, P, AROWS], F32, kind="ExternalInput").ap()
    bcat = nc.dram_tensor("bcat", [HID, N_CORE], F32, kind="ExternalInput").ap()
    cw = nc.dram_tensor("cw", [1, 1], F32, kind="ExternalInput").ap()
    out = nc.dram_tensor("out", [S_CORE, N_CORE], F32, kind="ExternalOutput").ap()

    xt_r = xt.rearrange("(kt p) t -> p kt t", p=P)
    wt_r = wt.rearrange("(kt p) n -> p kt n", p=P)

    with _DrainSplitTileContext(nc) as tc, ExitStack() as ctx:
        xres_p = ctx.enter_context(tc.tile_pool(name="xres", bufs=1))
        wch_p = ctx.enter_context(tc.tile_pool(name="wch", bufs=2))
        stage_p = ctx.enter_context(tc.tile_pool(name="stage", bufs=2))
        wstage_p = ctx.enter_context(tc.tile_pool(name="wstage", bufs=4))
        abf_p = ctx.enter_context(tc.tile_pool(name="abf", bufs=1))
        small_p = ctx.enter_context(tc.tile_pool(name="small", bufs=1))
        evict_p = ctx.enter_context(tc.tile_pool(name="evict", bufs=3))
        psb_p = ctx.enter_context(tc.tile_pool(name="psb", bufs=6, space="PSUM"))
        pss_p = ctx.enter_context(tc.tile_pool(name="pss", bufs=2, space="PSUM"))

        # ---- constants / small preloads ----
        a_bf = abf_p.tile([P, KT, AROWS], BF16)
        act_r = act.rearrange("kt p c -> p kt c")
        for half in range(2):
            a_st = stage_p.tile([P, KT // 2, AROWS], F32, tag="stage")
            ks = slice(half * KT // 2, (half + 1) * KT // 2)
            nc.sync.dma_start(out=a_st[:], in_=act_r[:, ks, :])
            nc.vector.tensor_copy(out=a_bf[:, ks, :], in_=a_st[:])

        bmat = small_p.tile([HID, N_CORE], F32)
        nc.sync.dma_start(out=bmat[:], in_=bcat)

        cwt = small_p.tile([1, 1], F32)
        nc.sync.dma_start(out=cwt[:], in_=cw)
        sig = small_p.tile([1, 1], F32)
        nc.scalar.activation(
            out=sig[:], in_=cwt[:], func=mybir.ActivationFunctionType.Sigmoid
        )
        cw2 = small_p.tile([1, 1], F32)
        nc.vector.tensor_scalar_mul(cw2[:], sig[:], SCALING)
        tsc = small_p.tile([1, 1], F32)  # (1 - sigmoid) * SCALING
        nc.vector.tensor_scalar(
            out=tsc[:], in0=sig[:], scalar1=-SCALING, scalar2=SCALING,
            op0=mybir.AluOpType.mult, op1=mybir.AluOpType.add,
        )

        # ---- x preload (fp32 -> bf16, SBUF resident) ----
        xres = xres_p.tile([P, KT, S_CORE], BF16)
        for kt in range(KT):
            xs = stage_p.tile([P, S_CORE], F32, tag="stage")
            nc.sync.dma_start(out=xs[:], in_=xt_r[:, kt, :])
            nc.vector.tensor_copy(out=xres[:, kt, :], in_=xs[:])

        # ---- phase A: hid^T = A_cat @ x^T ----
        hid = small_p.tile([HID, S_CORE], BF16)
        hacc = small_p.tile([AROWS, 1], F32)     # free-reduced phase-A rows
        hpart = small_p.tile([AROWS, 4], F32)
        for c in range(4):
            ph = pss_p.tile([AROWS, 512], F32, tag="pss")
            for kt in range(KT):
                nc.tensor.matmul(
                    ph[:], lhsT=a_bf[:, kt, :], rhs=xres[:, kt, c * 512:(c + 1) * 512],
                    start=(kt == 0), stop=(kt == KT - 1),
                )
            nc.vector.tensor_copy(out=hid[0:72, c * 512:(c + 1) * 512], in_=ph[0:72, :])
            nc.vector.tensor_reduce(
                out=hpart[:, c:c + 1], in_=ph[:], axis=mybir.AxisListType.X,
                op=mybir.AluOpType.add,
            )
        ones_s = small_p.tile([1, P], BF16)
        nc.vector.memset(ones_s[:], 1.0)
        for m in range(MT):
            nc.sync.dma_start(out=hid[72:73, m * P:(m + 1) * P], in_=ones_s[:])
        nc.vector.tensor_reduce(
            out=hacc[:], in_=hpart[:], axis=mybir.AxisListType.X,
            op=mybir.AluOpType.add,
        )

        # ---- routing ----
        l_row = small_p.tile([1, 8], F32)
        nc.sync.dma_start(out=l_row[:], in_=hacc[72:80, 0:1])  # partition->free
        e_row = small_p.tile([1, 8], F32)
        nc.scalar.activation(
            out=e_row[:], in_=l_row[:], func=mybir.ActivationFunctionType.Exp,
            scale=1.0 / S,
        )
        ssum = small_p.tile([1, 1], F32)
        nc.vector.tensor_reduce(
            out=ssum[:], in_=e_row[:], axis=mybir.AxisListType.X,
            op=mybir.AluOpType.add,
        )
        rec = small_p.tile([1, 1], F32)
        nc.vector.reciprocal(out=rec[:], in_=ssum[:])
        comb = small_p.tile([1, 1], F32)  # (1/sum) * (1-cw)*SCALING
        nc.vector.tensor_tensor(
            out=comb[:], in0=rec[:], in1=tsc[:], op=mybir.AluOpType.mult
        )
        ones8 = small_p.tile([1, 8], F32)
        nc.vector.memset(ones8[:], 1.0)
        svec_f = small_p.tile([1, HID], F32)
        nc.vector.tensor_scalar(
            out=svec_f[0:1, 0:8], in0=ones8[:], scalar1=cw2[:], scalar2=None,
            op0=mybir.AluOpType.mult,
        )
        for t in range(T):
            nc.vector.tensor_scalar(
                out=svec_f[0:1, 8 + 8 * t:16 + 8 * t], in0=ones8[:],
                scalar1=e_row[0:1, t:t + 1], scalar2=comb[:],
                op0=mybir.AluOpType.mult, op1=mybir.AluOpType.mult,
            )
        nc.vector.memset(svec_f[0:1, 72:73], 1.0)
        svec = small_p.tile([HID, 1], F32)
        nc.sync.dma_start(out=svec[:], in_=svec_f[:])  # free->partition
        bbf = small_p.tile([HID, N_CORE], BF16)
        nc.vector.tensor_scalar(
            out=bbf[:], in0=bmat[:], scalar1=svec[:], scalar2=None,
            op0=mybir.AluOpType.mult,
        )

        # ---- main loop: base matmul + fused down-proj ----
        for ncI in range(NNC):
            wch = wch_p.tile([P, KT, NCH], BF16)
            for kt in range(KT):
                ws = wstage_p.tile([P, NCH], F32)
                nc.sync.dma_start(
                    out=ws[:], in_=wt_r[:, kt, ncI * NCH:(ncI + 1) * NCH]
                )
                nc.vector.tensor_copy(out=wch[:, kt, :], in_=ws[:])
            for m in range(MT):
                ps = psb_p.tile([P, NCH], F32, tag="ps")
                for kt in range(KT):
                    nc.tensor.matmul(
                        ps[:], lhsT=xres[:, kt, m * P:(m + 1) * P], rhs=wch[:, kt, :],
                        start=(kt == 0), stop=False,
                    )
                nc.tensor.matmul(
                    ps[:], lhsT=hid[:, m * P:(m + 1) * P],
                    rhs=bbf[:, ncI * NCH:(ncI + 1) * NCH],
                    start=False, stop=True,
                )
                ev = evict_p.tile([P, NCH], F32)
                nc.scalar.activation(
                    out=ev[:], in_=ps[:], func=mybir.ActivationFunctionType.Copy
                )
                nc.scalar.dma_start(
                    out=out[m * P:(m + 1) * P, ncI * NCH:(ncI + 1) * NCH], in_=ev[:]
                )
    _split_multi_waits(nc)
    return nc


def prep_inputs(x, W, b, shared_A, shared_B, task_A, task_B, task_emb, collab_weight):
    """Host-side sharding/layout prep. Pure layout: slice/transpose/concat."""
    x = np.asarray(x, dtype=np.float32)
    W = np.asarray(W, dtype=np.float32)
    b = np.asarray(b, dtype=np.float32)
    a_cat = np.concatenate(
        [np.asarray(shared_A), np.asarray(task_A).reshape(T * R, DIN),
         np.asarray(task_emb)], axis=0
    ).astype(np.float32)                                   # [80, DIN]
    act = np.ascontiguousarray(a_cat.T.reshape(KT, P, AROWS))
    cwv = np.asarray(collab_weight, dtype=np.float32).reshape(1, 1)

    xt = [np.ascontiguousarray(x[p].T) for p in range(B)]  # [DIN, S] each
    wt, bc = [], []
    for h in range(2):
        cols = slice(h * N_CORE, (h + 1) * N_CORE)
        wt.append(np.ascontiguousarray(W[cols, :].T))      # [DIN, N_CORE]
        bcat = np.empty((HID, N_CORE), dtype=np.float32)
        bcat[0:8] = np.asarray(shared_B)[cols, :].T
        bcat[8:72] = np.asarray(task_B)[:, cols, :].transpose(0, 2, 1).reshape(
            T * R, N_CORE
        )
        bcat[72] = b[cols]
        bc.append(bcat)

    in_maps = []
    for i in range(N_CORES):
        p, h = i // 2, i % 2
        in_maps.append(
            {"xt": xt[p], "wt": wt[h], "act": act, "bcat": bc[h], "cw": cwv}
        )
    return in_maps


def assemble(results):
    out = np.empty((B, S, DOUT), dtype=np.float32)
    for i in range(N_CORES):
        p, h = i // 2, i % 2
        out[p, :, h * N_CORE:(h + 1) * N_CORE] = results[i]["out"]
    return out


_NC_CACHE = None


def kernel(**inputs) -> np.ndarray:
    global _NC_CACHE
    if _NC_CACHE is None:
        _NC_CACHE = build_nc()
    in_maps = prep_inputs(**inputs)
    res = run_bass_kernel_spmd(_NC_CACHE, in_maps, core_ids=list(range(N_CORES)))
    return assemble(res.results)
